# revision 1
# baseline (speedup 1.0000x reference)
"""GCN neighborhood mean-aggregation kernel for Trainium2 (8 NeuronCores).

Data-parallel over the batch of target nodes: the embedding table is
replicated to every core, nodes/neigh_idx are sharded along dim 0.  Each
core gathers its nodes' 33 rows (self + 32 sampled neighbors) via SWDGE
indirect DMA and mean-reduces them on VectorE.
"""

import numpy as np

from concourse import bass, bacc, mybir
import concourse.tile as tile
from concourse.bass_utils import run_bass_kernel_spmd

V, D = 100000, 128
B, K = 50000, 32
KP1 = K + 1  # 33 rows per node: self + neighbors
NCORES = 8
P = 128
NBLK = 49            # node blocks of 128 per core
BLOC = NBLK * P      # 6272 padded nodes per core
BPAD = BLOC * NCORES # 50176 >= B


def _build(nblk: int) -> bass.Bass:
    # idx DRAM layout is partition-major: idx[p, i*KP1 + k] holds the k-th
    # index of node i*128+p — the one-shot preload below is then a single
    # contiguous [128, nblk*KP1] DMA, and each block's offsets are a
    # contiguous per-partition slice of the persistent SBUF buffer.
    nc = bacc.Bacc(None)
    feats = nc.declare_dram_parameter(
        "features", [V, D], mybir.dt.float32, isOutput=False
    )
    idx = nc.declare_dram_parameter(
        "idx", [P, nblk * KP1], mybir.dt.int32, isOutput=False
    )
    out = nc.declare_dram_parameter(
        "out", [nblk * P, D], mybir.dt.float32, isOutput=True
    )

    with tile.TileContext(nc) as tc:
        with (
            tc.tile_pool(name="const", bufs=1) as cpool,
            tc.tile_pool(name="sbuf", bufs=3) as pool,
        ):
            idx_buf = cpool.tile([P, nblk * KP1], mybir.dt.int32)
            nc.sync.dma_start(out=idx_buf[:], in_=idx[:])
            for i in range(nblk):
                # 33 gathers per block: the HW indirect DMA consumes ONE
                # offset per partition per instruction, so gather k fetches
                # feats[idx[p, k]] into partition p's k-th row slot.
                gath = pool.tile([P, KP1 * D], mybir.dt.float32, tag="gath")
                for k in range(KP1):
                    nc.gpsimd.indirect_dma_start(
                        out=gath[:, k * D : (k + 1) * D],
                        out_offset=None,
                        in_=feats[:],
                        in_offset=bass.IndirectOffsetOnAxis(
                            ap=idx_buf[:, i * KP1 + k : i * KP1 + k + 1], axis=0
                        ),
                    )
                # Tree-reduce 33 rows into row block 0: fold row 32 in, then
                # halve 32 -> 16 -> 8 -> 4 -> 2 -> 1.
                nc.vector.tensor_add(
                    out=gath[:, 0:D], in0=gath[:, 0:D], in1=gath[:, 32 * D : 33 * D]
                )
                w = 16 * D
                while w >= D:
                    nc.vector.tensor_add(
                        out=gath[:, 0:w], in0=gath[:, 0:w], in1=gath[:, w : 2 * w]
                    )
                    w //= 2
                ot = pool.tile([P, D], mybir.dt.float32, tag="ot")
                nc.vector.tensor_scalar_mul(ot[:], gath[:, 0:D], 1.0 / KP1)
                nc.sync.dma_start(out=out[i * P : (i + 1) * P, :], in_=ot[:])
    nc.finalize()
    return nc


_CACHE: dict = {}

# test-harness knobs (the grading harness leaves these at defaults)
TRACE = False
LAST_RESULTS = None


def _get_nc() -> bass.Bass:
    if "nc" not in _CACHE:
        _CACHE["nc"] = _build(NBLK)
    return _CACHE["nc"]


def kernel(features, nodes, neigh_idx):
    feats = np.ascontiguousarray(np.asarray(features), dtype=np.float32)
    nodes = np.asarray(nodes)
    neigh = np.asarray(neigh_idx)

    idx_all = np.zeros((BPAD, KP1), dtype=np.int32)
    idx_all[:B, 0] = nodes.astype(np.int32)
    idx_all[:B, 1:] = neigh.astype(np.int32)
    # per-core partition-major layout: [NBLK, P, KP1] -> [P, NBLK*KP1]
    shards = (
        idx_all.reshape(NCORES, NBLK, P, KP1)
        .transpose(0, 2, 1, 3)
        .reshape(NCORES, P, NBLK * KP1)
    )

    nc = _get_nc()
    in_maps = [
        {"features": feats, "idx": np.ascontiguousarray(shards[c])}
        for c in range(NCORES)
    ]
    res = run_bass_kernel_spmd(nc, in_maps, list(range(NCORES)), trace=TRACE)
    global LAST_RESULTS
    LAST_RESULTS = res
    out = np.concatenate([res.results[c]["out"] for c in range(NCORES)], axis=0)
    return out[:B]



# revision 3
# speedup vs baseline: 1.0648x; 1.0648x over previous
"""GCN neighborhood mean-aggregation kernel for Trainium2 (8 NeuronCores).

Data-parallel over target nodes with batched SWDGE gathers.  The per-row
indirect-DMA baseline pays ~1.4us of serialized descriptor-generation per
128 rows; dma_gather amortizes that over thousands of rows per instruction
but takes int16 indices, so the 100000-row table is split into 4 chunks of
<=32768 rows (each prefixed with a zero pad row).  Nodes are sorted by
their per-chunk neighbor-count profile and grouped into 128-node blocks so
that the fixed per-block slot budgets (max count within the block) stay
close to the 33-row ideal; pad slots gather the chunk's zero row and fall
out of the mean for free.  Budgets are baked into the compiled NEFF and
cached per budget signature.
"""

import numpy as np

from concourse import bass, bacc, mybir
import concourse.tile as tile
from concourse.bass_utils import run_bass_kernel_spmd

V, D = 100000, 128
B, K = 50000, 32
KP1 = K + 1          # 33 rows per node: self + neighbors
NCORES = 8
P = 128
CH = 32767           # original rows per chunk
NCHUNK = 4           # 4*32767 >= 100000
CROWS = 32768        # chunk stride in the padded table (row 0 = zeros)
NB = 392             # total 128-node blocks (50176 slots >= B)
NBLK = NB // NCORES  # 49 blocks per core
NBG = 2              # blocks per gather-instruction group (per core)
NG = (NBLK + NBG - 1) // NBG  # 25 groups (24 full + 1 single-block)
INV_KP1 = 1.0 / KP1


def _group_blocks(g):
    """Number of blocks in group g (last group may be partial)."""
    return min(NBG, NBLK - g * NBG)


def _sorted_block(g, j, c):
    """Global sorted-block index handled by core c at (group g, slot j)."""
    return g * (NBG * NCORES) + j * NCORES + c


def _build(budgets):
    """budgets[g][j][cc] = slot budget shared by the 8 cores at (g, j)."""
    # derived static layout
    group_slots = []   # per g: total gathered slots across chunks
    chunk_offs = []    # per g: free-slot offset of each chunk region
    idx_offs = []      # per (g, cc): int16 free offset into the idx buffer
    idx_off = 0
    for g in range(NG):
        offs = []
        off = 0
        goffs = []
        for cc in range(NCHUNK):
            s_cc = sum(budgets[g][j][cc] for j in range(_group_blocks(g)))
            offs.append(off)
            goffs.append(idx_off)
            off += s_cc
            idx_off += (s_cc * P) // 16
        chunk_offs.append(offs)
        idx_offs.append(goffs)
        group_slots.append(off)
    idx_free = max(idx_off, 16)
    smax = max(max(group_slots), 1)

    nc = bacc.Bacc(None)
    feats = nc.declare_dram_parameter(
        "features", [NCHUNK * CROWS, D], mybir.dt.float32, isOutput=False
    )
    idx = nc.declare_dram_parameter(
        "idx", [P, idx_free], mybir.dt.int16, isOutput=False
    )
    out = nc.declare_dram_parameter(
        "out", [NBLK * P, D], mybir.dt.float32, isOutput=True
    )

    with tile.TileContext(nc) as tc:
        with (
            tc.tile_pool(name="const", bufs=1) as cpool,
            tc.tile_pool(name="sbuf", bufs=2) as pool,
        ):
            idx_buf = cpool.tile([P, idx_free], mybir.dt.int16)
            nc.sync.dma_start(out=idx_buf[:], in_=idx[:])
            for g in range(NG):
                nb = _group_blocks(g)
                S = group_slots[g]
                if S == 0:
                    continue
                gt = pool.tile([P, smax * D], mybir.dt.float32, tag="gath")
                for cc in range(NCHUNK):
                    s_cc = sum(budgets[g][j][cc] for j in range(nb))
                    if s_cc == 0:
                        continue
                    nidx = s_cc * P
                    dst = gt[
                        :, chunk_offs[g][cc] * D : (chunk_offs[g][cc] + s_cc) * D
                    ].rearrange("p (s e) -> p s e", e=D)
                    nc.gpsimd.dma_gather(
                        dst,
                        feats[cc * CROWS : (cc + 1) * CROWS, :],
                        idx_buf[:, idx_offs[g][cc] : idx_offs[g][cc] + nidx // 16],
                        nidx,
                        nidx,
                        D,
                        # one SDMA packet per descriptor: the coalesced
                        # single-packet stream is capped at 64 descriptors
                        # per engine lane and large gathers exceed it
                        single_packet=False,
                    )
                # per-(chunk, block) tree-reduce in place, then combine
                acc = pool.tile([P, NBG * D], mybir.dt.float32, tag="acc")
                for j in range(nb):
                    parts = []
                    for cc in range(NCHUNK):
                        t = budgets[g][j][cc]
                        if t == 0:
                            continue
                        a = chunk_offs[g][cc] + sum(
                            budgets[g][jp][cc] for jp in range(j)
                        )
                        vj = gt[:, a * D : (a + t) * D].rearrange(
                            "p (s e) -> p s e", e=D
                        )
                        while t > 1:
                            m = t // 2
                            nc.vector.tensor_add(
                                out=vj[:, 0:m, :],
                                in0=vj[:, 0:m, :],
                                in1=vj[:, t - m : t, :],
                            )
                            t -= m
                        parts.append(vj[:, 0, :])
                    aj = acc[:, j * D : (j + 1) * D]
                    if not parts:
                        nc.vector.memset(aj, 0.0)
                    elif len(parts) == 1:
                        nc.vector.tensor_copy(out=aj, in_=parts[0])
                    else:
                        nc.vector.tensor_add(out=aj, in0=parts[0], in1=parts[1])
                        for p_ in parts[2:]:
                            nc.vector.tensor_add(out=aj, in0=aj, in1=p_)
                ot = pool.tile([P, NBG * D], mybir.dt.float32, tag="ot")
                nc.vector.tensor_scalar_mul(
                    ot[:, : nb * D], acc[:, : nb * D], INV_KP1
                )
                dst = out[g * NBG * P : (g * NBG + nb) * P, :].rearrange(
                    "(j p) e -> p j e", p=P
                )
                nc.sync.dma_start(
                    out=dst, in_=ot[:, : nb * D].rearrange("p (j e) -> p j e", e=D)
                )
    nc.finalize()
    return nc


_CACHE: dict = {}

# test-harness knobs (the grading harness leaves these at defaults)
TRACE = False
LAST_RESULTS = None


def _prep(nodes, neigh):
    """Sort nodes by chunk-count profile, derive budgets and packed indices."""
    idx_all = np.concatenate(
        [nodes.astype(np.int64)[:, None], neigh.astype(np.int64)], axis=1
    ).astype(np.int32)  # [B, 33]
    chunk = idx_all // CH                     # [B, 33] in 0..3
    within = (idx_all - chunk * CH + 1).astype(np.int16)  # 1..32767

    counts = np.zeros((B, NCHUNK), np.int32)
    for c in range(NCHUNK):
        counts[:, c] = (chunk == c).sum(1)

    order = np.lexsort((counts[:, 3], counts[:, 2], counts[:, 1], counts[:, 0]))

    # per node: within-chunk indices grouped by chunk (stable sort over 33)
    o33 = np.argsort(chunk, axis=1, kind="stable")
    within_sorted = np.take_along_axis(within, o33, axis=1)  # [B, 33]
    starts = np.zeros((B, NCHUNK), np.int32)
    starts[:, 1:] = np.cumsum(counts, axis=1)[:, :-1]

    # padded per-node per-chunk row matrix [NPAD, 4, tmax]
    tmax = int(counts.max())
    NPAD = NB * P
    padval = np.zeros((NPAD, NCHUNK, tmax), np.int16)
    s_idx = np.arange(tmax)[None, None, :]
    mask = s_idx < counts[:, :, None]
    gather_pos = np.minimum(starts[:, :, None] + s_idx, KP1 - 1)
    vals = np.take_along_axis(
        within_sorted, gather_pos.reshape(B, -1), axis=1
    ).reshape(B, NCHUNK, tmax)
    padval[:B] = np.where(mask, vals, 0)[order][: B]
    cpad = np.zeros((NPAD, NCHUNK), np.int32)
    cpad[:B] = counts[order]

    # budgets: shared across the 8 cores at each (g, j)
    blocks = cpad.reshape(NB, P, NCHUNK).max(1)  # per-block maxima
    budgets = []
    for g in range(NG):
        nb = _group_blocks(g)
        row = []
        for j in range(nb):
            s0 = _sorted_block(g, j, 0)
            row.append(tuple(int(x) for x in blocks[s0 : s0 + NCORES].max(0)))
        budgets.append(tuple(row))
    budgets = tuple(budgets)

    # pack per-core idx streams
    idx_free = 0
    for g in range(NG):
        for cc in range(NCHUNK):
            s_cc = sum(budgets[g][j][cc] for j in range(_group_blocks(g)))
            idx_free += (s_cc * P) // 16
    idx_free = max(idx_free, 16)

    idx_streams = np.zeros((NCORES, P, idx_free), np.int16)
    nodes_of = order  # sorted slot -> node id (slots >= B are dummies)
    for c in range(NCORES):
        off = 0
        for g in range(NG):
            nb = _group_blocks(g)
            for cc in range(NCHUNK):
                segs = []
                for j in range(nb):
                    t = budgets[g][j][cc]
                    if t == 0:
                        continue
                    s = _sorted_block(g, j, c)
                    sl = slice(s * P, (s + 1) * P)
                    segs.append(padval[sl, cc, :t].T)  # [t, 128]
                if not segs:
                    continue
                arr = np.concatenate(segs, axis=0).reshape(-1)  # positions
                n16 = arr.size // 16
                packed = arr.reshape(n16, 16).T  # [16, n16]
                idx_streams[c, :, off : off + n16] = np.tile(packed, (8, 1))
                off += n16
    return budgets, idx_streams, order


def kernel(features, nodes, neigh_idx):
    feats = np.asarray(features)
    nodes = np.asarray(nodes)
    neigh = np.asarray(neigh_idx)

    # chunked table: row 0 of each chunk is a zero pad row
    feats2 = np.zeros((NCHUNK * CROWS, D), dtype=np.float32)
    for c in range(NCHUNK):
        lo = c * CH
        hi = min(lo + CH, V)
        feats2[c * CROWS + 1 : c * CROWS + 1 + (hi - lo)] = feats[lo:hi]

    budgets, idx_streams, order = _prep(nodes, neigh)

    key = budgets
    if key not in _CACHE:
        _CACHE.clear()
        _CACHE[key] = _build(budgets)
    nc = _CACHE[key]

    in_maps = [
        {"features": feats2, "idx": np.ascontiguousarray(idx_streams[c])}
        for c in range(NCORES)
    ]
    res = run_bass_kernel_spmd(nc, in_maps, list(range(NCORES)), trace=TRACE)
    global LAST_RESULTS
    LAST_RESULTS = res

    # un-permute: sorted slot (s, p) lives at core (s % 8 at its position)
    big = np.empty((NB * P, D), dtype=np.float32)
    for c in range(NCORES):
        r = res.results[c]["out"]  # [NBLK*128, 128]
        for g in range(NG):
            nb = _group_blocks(g)
            for j in range(nb):
                s = _sorted_block(g, j, c)
                rr = (g * NBG + j) * P
                big[s * P : (s + 1) * P] = r[rr : rr + P]
    out = np.empty((B, D), dtype=np.float32)
    out[order] = big[:B]
    return out


# revision 15
# speedup vs baseline: 1.0673x; 1.0024x over previous
"""GCN neighborhood mean-aggregation kernel for Trainium2 (8 NeuronCores).

Data-parallel over target nodes with batched SWDGE dma_gather.  The per-row
indirect-DMA baseline pays ~1.45us of serialized Pool-engine time per 128
rows (one offset per partition per instruction); dma_gather amortizes that
over thousands of rows per instruction (measured Q7 descriptor generation:
~0.8us + 7.8ns/row, the hard bottleneck for random row gathers).

dma_gather takes int16 indices, so the 100000-row table is split into 4
chunks of <=32767 rows (each prefixed with a zero pad row).  Nodes are
sorted by their per-chunk neighbor-count profile and grouped into 128-node
blocks; the per-chunk slot budget of the 8 sorted blocks sharing a program
position (SPMD: one NEFF for all cores) is their max per-node chunk count.
Pad slots gather the chunk's zero row, which drops out of the mean.  The
budgets are input-derived and baked into the compiled NEFF (cached per
budget signature; host-side prep and compile are not part of HW exec time).
"""

import numpy as np

from concourse import bass, bacc, mybir
import concourse.tile as tile
from concourse.bass_utils import run_bass_kernel_spmd

V, D = 100000, 128
B, K = 50000, 32
KP1 = K + 1          # 33 rows per node: self + neighbors
NCORES = 8
P = 128
CH = 32767           # original rows per chunk
NCHUNK = 4           # 4*32767 >= 100000
CROWS = 32768        # chunk stride in the padded table (row 0 = zeros)
NB = 392             # total 128-node blocks (50176 slots >= B)
NBLK = NB // NCORES  # 49 blocks per core
NBG = 2              # blocks per gather-instruction group (per core)
NG = (NBLK + NBG - 1) // NBG  # 25 groups (24 full + 1 single-block)
INV_KP1 = 1.0 / KP1


def _group_blocks(g):
    """Number of blocks in group g (last group may be partial)."""
    return min(NBG, NBLK - g * NBG)


def _sorted_block(g, j, c):
    """Global sorted-block index handled by core c at (group g, slot j)."""
    return g * (NBG * NCORES) + j * NCORES + c


def _build(budgets):
    """budgets[g][j][cc] = slot budget shared by the 8 cores at (g, j)."""
    group_slots = []   # per g: total gathered slots across chunks
    chunk_offs = []    # per g: free-slot offset of each chunk region
    idx_offs = []      # per (g, cc): int16 free offset into the idx buffer
    idx_off = 0
    for g in range(NG):
        offs = []
        off = 0
        goffs = []
        for cc in range(NCHUNK):
            s_cc = sum(budgets[g][j][cc] for j in range(_group_blocks(g)))
            offs.append(off)
            goffs.append(idx_off)
            off += s_cc
            idx_off += (s_cc * P) // 16
        chunk_offs.append(offs)
        idx_offs.append(goffs)
        group_slots.append(off)
    idx_free = max(idx_off, 16)
    smax = max(max(group_slots), 1)

    nc = bacc.Bacc(None)
    feats = nc.declare_dram_parameter(
        "features", [NCHUNK * CROWS, D], mybir.dt.float32, isOutput=False
    )
    idx = nc.declare_dram_parameter(
        "idx", [P, idx_free], mybir.dt.int16, isOutput=False
    )
    out = nc.declare_dram_parameter(
        "out", [NBLK * P, D], mybir.dt.float32, isOutput=True
    )

    with tile.TileContext(nc) as tc:
        with (
            tc.tile_pool(name="const", bufs=1) as cpool,
            tc.tile_pool(name="sbuf", bufs=2) as pool,
        ):
            idx_buf = cpool.tile([P, idx_free], mybir.dt.int16)
            nc.sync.dma_start(out=idx_buf[:], in_=idx[:])
            for g in range(NG):
                nb = _group_blocks(g)
                S = group_slots[g]
                if S == 0:
                    continue
                gt = pool.tile([P, smax * D], mybir.dt.float32, tag="gath")
                for cc in range(NCHUNK):
                    s_cc = sum(budgets[g][j][cc] for j in range(nb))
                    if s_cc == 0:
                        continue
                    nidx = s_cc * P
                    dst = gt[
                        :, chunk_offs[g][cc] * D : (chunk_offs[g][cc] + s_cc) * D
                    ].rearrange("p (s e) -> p s e", e=D)
                    nc.gpsimd.dma_gather(
                        dst,
                        feats[cc * CROWS : (cc + 1) * CROWS, :],
                        idx_buf[:, idx_offs[g][cc] : idx_offs[g][cc] + nidx // 16],
                        nidx,
                        nidx,
                        D,
                        # one SDMA packet per descriptor: the coalesced
                        # single-packet stream is capped at 64 descriptors
                        # per engine lane and large gathers exceed it
                        single_packet=False,
                    )
                # per-(chunk, block) tree-reduce in place, then combine
                acc = pool.tile([P, NBG * D], mybir.dt.float32, tag="acc")
                for j in range(nb):
                    parts = []
                    for cc in range(NCHUNK):
                        t = budgets[g][j][cc]
                        if t == 0:
                            continue
                        a = chunk_offs[g][cc] + sum(
                            budgets[g][jp][cc] for jp in range(j)
                        )
                        vj = gt[:, a * D : (a + t) * D].rearrange(
                            "p (s e) -> p s e", e=D
                        )
                        while t > 1:
                            m = t // 2
                            nc.vector.tensor_add(
                                out=vj[:, 0:m, :],
                                in0=vj[:, 0:m, :],
                                in1=vj[:, t - m : t, :],
                            )
                            t -= m
                        parts.append(vj[:, 0, :])
                    aj = acc[:, j * D : (j + 1) * D]
                    if not parts:
                        nc.vector.memset(aj, 0.0)
                    elif len(parts) == 1:
                        nc.vector.tensor_copy(out=aj, in_=parts[0])
                    else:
                        nc.vector.tensor_add(out=aj, in0=parts[0], in1=parts[1])
                        for p_ in parts[2:]:
                            nc.vector.tensor_add(out=aj, in0=aj, in1=p_)
                ot = pool.tile([P, NBG * D], mybir.dt.float32, tag="ot")
                nc.vector.tensor_scalar_mul(
                    ot[:, : nb * D], acc[:, : nb * D], INV_KP1
                )
                dst = out[g * NBG * P : (g * NBG + nb) * P, :].rearrange(
                    "(j p) e -> p j e", p=P
                )
                nc.sync.dma_start(
                    out=dst, in_=ot[:, : nb * D].rearrange("p (j e) -> p j e", e=D)
                )
    nc.finalize()
    return nc


_CACHE: dict = {}

# test-harness knobs (the grading harness leaves these at defaults)
TRACE = False
LAST_RESULTS = None


def _prep(nodes, neigh):
    """Sort nodes by chunk-count profile, derive budgets and packed indices."""
    idx_all = np.concatenate(
        [nodes.astype(np.int64)[:, None], neigh.astype(np.int64)], axis=1
    ).astype(np.int32)  # [B, 33]
    chunk = idx_all // CH                     # [B, 33] in 0..3
    within = (idx_all - chunk * CH + 1).astype(np.int16)  # 1..32767

    counts = np.zeros((B, NCHUNK), np.int32)
    for c in range(NCHUNK):
        counts[:, c] = (chunk == c).sum(1)

    order = np.lexsort((counts[:, 3], counts[:, 2], counts[:, 1], counts[:, 0]))

    # per node: within-chunk indices grouped by chunk (stable sort over 33)
    o33 = np.argsort(chunk, axis=1, kind="stable")
    within_sorted = np.take_along_axis(within, o33, axis=1)  # [B, 33]
    starts = np.zeros((B, NCHUNK), np.int32)
    starts[:, 1:] = np.cumsum(counts, axis=1)[:, :-1]

    # padded per-node per-chunk row matrix [NPAD, 4, tmax]
    tmax = int(counts.max())
    NPAD = NB * P
    padval = np.zeros((NPAD, NCHUNK, tmax), np.int16)
    s_idx = np.arange(tmax)[None, None, :]
    mask = s_idx < counts[:, :, None]
    gather_pos = np.minimum(starts[:, :, None] + s_idx, KP1 - 1)
    vals = np.take_along_axis(
        within_sorted, gather_pos.reshape(B, -1), axis=1
    ).reshape(B, NCHUNK, tmax)
    padval[:B] = np.where(mask, vals, 0)[order]
    cpad = np.zeros((NPAD, NCHUNK), np.int32)
    cpad[:B] = counts[order]

    # budgets: shared across the 8 cores at each (g, j)
    blocks = cpad.reshape(NB, P, NCHUNK).max(1)  # per-block maxima
    budgets = []
    for g in range(NG):
        nb = _group_blocks(g)
        row = []
        for j in range(nb):
            s0 = _sorted_block(g, j, 0)
            row.append(tuple(int(x) for x in blocks[s0 : s0 + NCORES].max(0)))
        budgets.append(tuple(row))
    budgets = tuple(budgets)

    # pack per-core idx streams
    idx_free = 0
    for g in range(NG):
        for cc in range(NCHUNK):
            s_cc = sum(budgets[g][j][cc] for j in range(_group_blocks(g)))
            idx_free += (s_cc * P) // 16
    idx_free = max(idx_free, 16)

    idx_streams = np.zeros((NCORES, P, idx_free), np.int16)
    for c in range(NCORES):
        off = 0
        for g in range(NG):
            nb = _group_blocks(g)
            for cc in range(NCHUNK):
                segs = []
                for j in range(nb):
                    t = budgets[g][j][cc]
                    if t == 0:
                        continue
                    s = _sorted_block(g, j, c)
                    sl = slice(s * P, (s + 1) * P)
                    segs.append(padval[sl, cc, :t].T)  # [t, 128]
                if not segs:
                    continue
                arr = np.concatenate(segs, axis=0).reshape(-1)  # positions
                n16 = arr.size // 16
                packed = arr.reshape(n16, 16).T  # [16, n16]
                idx_streams[c, :, off : off + n16] = np.tile(packed, (8, 1))
                off += n16
    return budgets, idx_streams, order


def kernel(features, nodes, neigh_idx):
    feats = np.asarray(features)
    nodes = np.asarray(nodes)
    neigh = np.asarray(neigh_idx)

    # chunked table: row 0 of each chunk is a zero pad row
    feats2 = np.zeros((NCHUNK * CROWS, D), dtype=np.float32)
    for c in range(NCHUNK):
        lo = c * CH
        hi = min(lo + CH, V)
        feats2[c * CROWS + 1 : c * CROWS + 1 + (hi - lo)] = feats[lo:hi]

    budgets, idx_streams, order = _prep(nodes, neigh)

    key = budgets
    if key not in _CACHE:
        _CACHE.clear()
        _CACHE[key] = _build(budgets)
    nc = _CACHE[key]

    in_maps = [
        {"features": feats2, "idx": np.ascontiguousarray(idx_streams[c])}
        for c in range(NCORES)
    ]
    res = run_bass_kernel_spmd(nc, in_maps, list(range(NCORES)), trace=TRACE)
    global LAST_RESULTS
    LAST_RESULTS = res

    # un-permute: sorted slot (s, p) lives at core (s % 8 at its position)
    big = np.empty((NB * P, D), dtype=np.float32)
    for c in range(NCORES):
        r = res.results[c]["out"]  # [NBLK*128, 128]
        for g in range(NG):
            nb = _group_blocks(g)
            for j in range(nb):
                s = _sorted_block(g, j, c)
                rr = (g * NBG + j) * P
                big[s * P : (s + 1) * P] = r[rr : rr + P]
    out = np.empty((B, D), dtype=np.float32)
    out[order] = big[:B]
    return out
